# revision 4
# baseline (speedup 1.0000x reference)
"""Distributed real SHT (spherical harmonic transform) on 8 trn2 NeuronCores.

Pipeline:
  out[b,c,l,m] = sum_k W[m,l,k] * XF[b,c,m,k],   XF = (2*pi/nlon) * rfft(x, lon)[..., :mmax]

Stage A (launch 1, channel-sharded): DFT along longitude as bf16 matmuls with a
DOUBLE longitude fold: n -> n' (rfft realness, cos/sin split) and n' -> n''
(reflection about nlon/4) which splits m by parity. Four quadrant DFTs
(cos-even, cos-odd, sin-even, sin-odd) of ~181x181 replace two 361x361 ones.
The DFT matrices are the STATIONARY operand (psum partitions = m) so the
moving operand streams 362 latitude columns per matmul -- long enough to hide
LDWEIGHTS, which a short-stream layout exposes (~148ns/matmul floor). The
m>=256 half-tiles stream only the latitude window where any P_l^m (m>=256) is
non-negligible.

Host exchange (free): unfold quadrants to XF[c,k,m], fold latitude using
P_l^m(pi-th) = (-1)^(l+m) P_l^m(th)  ->  xe/xo parts on k' in [0,181).

Stage B (launch 2, m-sharded, m interleaved mod 8): folded Legendre contraction
  psum[l_tile, (ri,c)=512] += W[m, l_par, k'_chunk]^T @ x_par[k'_chunk, 512]
over the exact per-group latitude window [klo, 181). Both l-parities
(even/odd l-m -> xe/xo) are computed per m into one 2-bank psum pair tile so a
single wide cast drains both. rhs and weights live in one dram tensor but load
into SEPARATE tiles (weights-as-stationary needs its own tile to get the
double-buffered LDWEIGHTS path; same-tile operands fall into a ~250ns/matmul
slower fused path).

DMA packet cost is ~9ns + bytes/(20-27GB/s) per partition-line across 16
engines, so every DRAM layout here is packed for >=1-3KB contiguous lines.
bf16 operands; psum accumulation is fp32. fp8 was measured to break the 2e-2
error budget (2.7e-2 with fp8 XF alone), so everything stays bf16.
"""

import os

import numpy as np

import concourse.bacc as bacc
import concourse.mybir as mybir
from concourse.tile import TileContext
from concourse.bass_utils import run_bass_kernel_spmd

LAST_PERF = {}

NLAT = 361
NLON = 720
MMAX = 361
LMAX = 361
C = 256
NCORES = 8
CPC = C // NCORES  # 32 channels per core
MPC = (MMAX + NCORES - 1) // NCORES  # 46 m's per core (padded)

MH = 182  # m-half columns, padded even (ce:181, co:180, se:181, so:180)
KCOLS = 362  # nlat padded even
NQR = [181, 180, 179, 180]  # quadrant fold-row counts (ce, co, se, so)
NP2 = [q - 128 for q in NQR]  # partial contraction-chunk rows [53, 52, 51, 52]
MW1 = [53, 52, 53, 52]  # m-cols of the second m-tile per quadrant
KHALF = 181  # folded latitude rows

F32 = mybir.dt.float32
BF16 = mybir.dt.bfloat16


# ---------------------------------------------------------------- stage A ----


def build_stage_a(klo2, kw2e, cpc=CPC):
    """xin [cpc, 181, 4*KCOLS] bf16: rows 0:128 = contraction chunk0 of each
    quadrant, rows 128:181 = partial chunk1 (quadrant q at cols q*KCOLS).
    mats [181, 4*MH]: same row split; quadrant q's DFT matrix columns (m-half)
    at q*MH.  aoutA [cpc, 128, 4*KCOLS]: m-tile0 (m-half rows 0:128) x full k.
    aoutB [cpc, 53, 4*kw2e]: m-tile1 (m-half rows 128:) x k in [klo2, klo2+kw2e)."""
    nc = bacc.Bacc("TRN2", target_bir_lowering=False)
    xin = nc.dram_tensor("xin", [cpc, 181, 4 * KCOLS], BF16, kind="ExternalInput")
    mats = nc.dram_tensor("mats", [181, 4 * MH], BF16, kind="ExternalInput")
    aoutA = nc.dram_tensor("aoutA", [cpc, 128, 4 * KCOLS], BF16, kind="ExternalOutput")
    aoutB = nc.dram_tensor("aoutB", [cpc, 53, 4 * kw2e], BF16, kind="ExternalOutput")

    cast_idx = 0
    with TileContext(nc) as tc:
        with (
            tc.tile_pool(name="mats", bufs=1) as matp,
            tc.tile_pool(name="xinp", bufs=3) as xinp,
            tc.tile_pool(name="outp", bufs=3) as outp,
            tc.tile_pool(name="ps", bufs=4, space="PSUM") as psp,
        ):
            mt1 = matp.tile([128, 4 * MH], BF16, tag="m1")
            mt2 = matp.tile([64, 4 * MH], BF16, tag="m2")
            nc.sync.dma_start(out=mt1, in_=mats[:128, :])
            nc.sync.dma_start(out=mt2[:53], in_=mats[128:181, :])
            for c in range(cpc):
                xt1 = xinp.tile([128, 4 * KCOLS], BF16, tag="x1")
                xt2 = xinp.tile([64, 4 * KCOLS], BF16, tag="x2")
                nc.sync.dma_start(out=xt1, in_=xin[c, :128, :])
                nc.sync.dma_start(out=xt2[:53], in_=xin[c, 128:181, :])
                otA = outp.tile([128, 4 * KCOLS], BF16, tag="oA")
                otB = outp.tile([53, 4 * kw2e], BF16, tag="oB")
                for q in range(4):
                    np2 = NP2[q]
                    # m-tile 0: full latitude stream
                    ps0 = psp.tile([128, KCOLS], F32, tag="ps0")
                    nc.tensor.matmul(
                        ps0[:, :],
                        mt1[:, q * MH : q * MH + 128],
                        xt1[:, q * KCOLS : q * KCOLS + KCOLS],
                        start=True,
                        stop=False,
                    )
                    nc.tensor.matmul(
                        ps0[:, :],
                        mt2[:np2, q * MH : q * MH + 128],
                        xt2[:np2, q * KCOLS : q * KCOLS + KCOLS],
                        start=False,
                        stop=True,
                    )
                    # m-tile 1 (m >= 256): windowed latitude stream
                    mw = MW1[q]
                    ps1 = psp.tile([53, kw2e], F32, tag="ps1")
                    nc.tensor.matmul(
                        ps1[:mw, :],
                        mt1[:, q * MH + 128 : q * MH + 128 + mw],
                        xt1[:, q * KCOLS + klo2 : q * KCOLS + klo2 + kw2e],
                        start=True,
                        stop=False,
                    )
                    nc.tensor.matmul(
                        ps1[:mw, :],
                        mt2[:np2, q * MH + 128 : q * MH + 128 + mw],
                        xt2[:np2, q * KCOLS + klo2 : q * KCOLS + klo2 + kw2e],
                        start=False,
                        stop=True,
                    )
                    dstA = otA[:, q * KCOLS : (q + 1) * KCOLS]
                    dstB = otB[:mw, q * kw2e : q * kw2e + kw2e]
                    for dst, src in ((dstA, ps0[:, :]), (dstB, ps1[:mw, :])):
                        if cast_idx % 2 == 0:
                            nc.vector.tensor_copy(out=dst, in_=src)
                        else:
                            nc.scalar.copy(dst, src)
                        cast_idx += 1
                nc.gpsimd.dma_start(out=aoutA[c], in_=otA)
                nc.gpsimd.dma_start(out=aoutB[c], in_=otB)
    nc.compile()
    return nc


def _quadrant_mats():
    """DFT quadrant matrices [n'' rows, m-half cols], 2*pi/nlon scale folded.
    Verified exact against np.fft.rfft."""
    s = 2.0 * np.pi / NLON
    m = np.arange(MMAX)
    npp = np.arange(181)
    ang = 2.0 * np.pi * np.outer(npp, m % NLON) / NLON
    cosm = s * np.cos(ang)  # [n''=0..180, m]
    sinm = -s * np.sin(ang)
    mat_ce = cosm[:181][:, m % 2 == 0]  # 181 x 181
    mat_co = cosm[:180][:, m % 2 == 1]  # 180 x 180
    mat_se = sinm[1:180][:, m % 2 == 0]  # 179 x 181
    mat_so = sinm[1:181][:, m % 2 == 1]  # 180 x 180
    return [mat_ce, mat_co, mat_se, mat_so]


def fold_quadrants(x):
    """x: (C, nlat, nlon) f32 -> 4 arrays (C, nlat, rows_q): folded inputs for
    the quadrant DFTs (ce, co, se, so)."""
    xc = np.empty((x.shape[0], x.shape[1], 361), dtype=np.float32)
    xc[..., 0] = x[..., 0]
    xc[..., 360] = x[..., 360]
    xc[..., 1:360] = x[..., 1:360] + x[..., :360:-1]
    xsf = np.zeros((x.shape[0], x.shape[1], 361), dtype=np.float32)
    xsf[..., 1:360] = x[..., 1:360] - x[..., :360:-1]

    xce = np.empty((x.shape[0], x.shape[1], 181), dtype=np.float32)
    xce[..., :180] = xc[..., :180] + xc[..., 360:180:-1]
    xce[..., 180] = xc[..., 180]
    xco = xc[..., :180] - xc[..., 360:180:-1]
    xse = xsf[..., 1:180] - xsf[..., 359:180:-1]
    xso = np.empty((x.shape[0], x.shape[1], 180), dtype=np.float32)
    xso[..., :179] = xsf[..., 1:180] + xsf[..., 359:180:-1]
    xso[..., 179] = xsf[..., 180]
    return [xce, xco, xse, xso]


def pack_stage_a_inputs(x):
    """x: (C, nlat, nlon) f32 -> xin (C, 181, 4*KCOLS) bf16, mats (181, 4*MH)
    bf16."""
    import ml_dtypes

    bf = ml_dtypes.bfloat16
    quads = fold_quadrants(x)
    xin = np.zeros((x.shape[0], 181, 4 * KCOLS), dtype=bf)
    for q, xq in enumerate(quads):
        nq = NQR[q]
        xt = xq.transpose(0, 2, 1)  # (C, rows, k)
        xin[:, :128, q * KCOLS : q * KCOLS + NLAT] = xt[:, :128].astype(bf)
        xin[:, 128 : 128 + NP2[q], q * KCOLS : q * KCOLS + NLAT] = xt[:, 128:nq].astype(
            bf
        )
    mats = np.zeros((181, 4 * MH), dtype=bf)
    for q, mq in enumerate(_quadrant_mats()):
        nq, ncol = mq.shape
        mats[:128, q * MH : q * MH + ncol] = mq[:128].astype(bf)
        mats[128 : 128 + NP2[q], q * MH : q * MH + ncol] = mq[128:nq].astype(bf)
    return xin, mats


def unpack_stage_a(results, klo2, kw2):
    """-> re, im arrays (C, nlat, mmax) f32.  XF for m>=256 outside the
    [klo2, klo2+kw2) latitude window is left zero (stage B never reads it)."""
    arrA = np.concatenate(
        [np.asarray(r["aoutA"], dtype=np.float32) for r in results], axis=0
    ).reshape(C, 128, 4, KCOLS)
    arrB = np.concatenate(
        [np.asarray(r["aoutB"], dtype=np.float32) for r in results], axis=0
    )
    kw2e = arrB.shape[-1] // 4
    arrB = arrB.reshape(C, 53, 4, kw2e)
    re = np.zeros((C, NLAT, MMAX), dtype=np.float32)
    im = np.zeros((C, NLAT, MMAX), dtype=np.float32)
    # m-tile 0: m-half rows 0..127 -> m = 2r (+1), all k
    re[:, :, 0:256:2] = arrA[:, :, 0, :NLAT].transpose(0, 2, 1)
    re[:, :, 1:256:2] = arrA[:, :, 1, :NLAT].transpose(0, 2, 1)
    im[:, :, 0:256:2] = arrA[:, :, 2, :NLAT].transpose(0, 2, 1)
    im[:, :, 1:256:2] = arrA[:, :, 3, :NLAT].transpose(0, 2, 1)
    # m-tile 1: m-half rows 128.. -> m = 256+2r (+1), k in [klo2, klo2+kw2)
    ksl = slice(klo2, klo2 + kw2)
    re[:, ksl, 256::2] = arrB[:, :53, 0, :kw2].transpose(0, 2, 1)
    re[:, ksl, 257::2] = arrB[:, :52, 1, :kw2].transpose(0, 2, 1)
    im[:, ksl, 256::2] = arrB[:, :53, 2, :kw2].transpose(0, 2, 1)
    im[:, ksl, 257::2] = arrB[:, :52, 3, :kw2].transpose(0, 2, 1)
    return re, im


# ---------------------------------------------------------------- stage B ----


def _nlab(i):
    nl = LMAX - NCORES * i
    return (nl + 1) // 2, nl // 2


def b_order(mpc):
    """Interleave heavy (small i) and light (large i) iterations; lightest
    last so the post-matmul drain tail is minimal."""
    order = []
    lo, hi = 0, mpc - 2
    while lo <= hi:
        order.append(lo)
        if hi != lo:
            order.append(hi)
        lo += 1
        hi -= 1
    order.append(mpc - 1)
    return order


def build_stage_b(hw_list):
    """xw [MPC, 256, 1388] bf16: rows = folded-latitude window rows (k'-klo),
    cols 0:512 = rhs slot0 (re|im x 256ch), 512:1024 = slot1, 1024:1388 =
    weight l-columns [pass-A (nlA) | pass-B (nlB)].  bout [MPC, 2, 128, 1024]
    bf16: [i, l-tile, l-row, passA(512)|passB(512)]."""
    nc = bacc.Bacc("TRN2", target_bir_lowering=False)
    xw = nc.dram_tensor("xw", [MPC, 256, 1388], BF16, kind="ExternalInput")
    bout = nc.dram_tensor("bout", [MPC, 2, 128, 1024], BF16, kind="ExternalOutput")

    order = b_order(MPC)
    cast_idx = 0
    with TileContext(nc) as tc:
        with (
            tc.tile_pool(name="rh0", bufs=4) as rp0,
            tc.tile_pool(name="rh1", bufs=4) as rp1,
            tc.tile_pool(name="wt0", bufs=4) as wp0,
            tc.tile_pool(name="wt1", bufs=4) as wp1,
            tc.tile_pool(name="outp", bufs=4) as outp,
            tc.tile_pool(name="ps", bufs=3, space="PSUM") as psp,
        ):
            for bi in range(MPC):
                i = order[bi]
                hw = hw_list[i]
                rc = [min(128, hw), max(0, hw - 128)]
                nkc = 2 if rc[1] > 0 else 1
                r0 = rp0.tile([128, 1024], BF16, tag="r0")
                w0 = wp0.tile([128, 364], BF16, tag="w0")
                nc.sync.dma_start(out=r0[: rc[0]], in_=xw[i, : rc[0], :1024])
                nc.scalar.dma_start(out=w0[: rc[0]], in_=xw[i, : rc[0], 1024:1388])
                rts, wts = [r0], [w0]
                if nkc == 2:
                    r1 = rp1.tile([64, 1024], BF16, tag="r1")
                    w1 = wp1.tile([64, 364], BF16, tag="w1")
                    nc.sync.dma_start(out=r1[: rc[1]], in_=xw[i, 128 : 128 + rc[1], :1024])
                    nc.scalar.dma_start(
                        out=w1[: rc[1]], in_=xw[i, 128 : 128 + rc[1], 1024:1388]
                    )
                    rts.append(r1)
                    wts.append(w1)
                nlA, nlB = _nlab(i)
                for tp in range(-(-nlA // 128)):
                    ot = outp.tile([128, 1024], BF16, tag="ot")
                    ps = psp.tile([128, 1024], F32, tag="ps")
                    rows = 0
                    for s, (nls, coff) in enumerate([(nlA, 0), (nlB, nlA)]):
                        lp = min(128, nls - tp * 128)
                        if lp <= 0:
                            continue
                        for ck in range(nkc):
                            nc.tensor.matmul(
                                ps[:lp, s * 512 : (s + 1) * 512],
                                wts[ck][:rc[ck], coff + tp * 128 : coff + tp * 128 + lp],
                                rts[ck][:rc[ck], s * 512 : (s + 1) * 512],
                                start=(ck == 0),
                                stop=(ck == nkc - 1),
                            )
                        rows = max(rows, lp)
                    # one wide cast drains both parity passes; gpsimd cannot
                    # read PSUM, so split vector-heavy/scalar-light (scalar
                    # also issues the weight loads)
                    if cast_idx % 5 == 4:
                        nc.scalar.copy(ot[:rows, :], ps[:rows, :])
                    else:
                        nc.vector.tensor_copy(out=ot[:rows, :], in_=ps[:rows, :])
                    cast_idx += 1
                    nc.gpsimd.dma_start(out=bout[i, tp, :rows, :], in_=ot[:rows, :])
    nc.compile()
    return nc


def compute_windows(weights):
    """Folded-latitude window per m-group: klo_i = first k' (0..180) where any
    |W[m,l,k']| with m in group i is non-negligible; support always reaches the
    equator k'=180."""
    wabs = np.abs(weights).max(axis=1)  # (m, k)
    thr = 1e-7 * wabs.max()
    klo_m = np.empty(MMAX, dtype=np.int64)
    for m in range(MMAX):
        nz = np.nonzero(wabs[m, :KHALF] > thr)[0]
        klo_m[m] = nz[0] if len(nz) else KHALF - 1
    windows = []
    for i in range(MPC):
        ms = [NCORES * i + j for j in range(NCORES) if NCORES * i + j < MMAX]
        klo = int(min(klo_m[m] for m in ms))
        windows.append((klo, KHALF - klo))
    return windows


def pack_stage_b_inputs(re, im, weights, windows):
    """re/im: (C, nlat, mmax) f32.  -> per-core xw tensors (bf16)."""
    import ml_dtypes

    bf = ml_dtypes.bfloat16
    # latitude fold (host, f32)
    xe_re = np.empty((C, KHALF, MMAX), dtype=np.float32)
    xe_im = np.empty((C, KHALF, MMAX), dtype=np.float32)
    xo_re = np.zeros((C, KHALF, MMAX), dtype=np.float32)
    xo_im = np.zeros((C, KHALF, MMAX), dtype=np.float32)
    xe_re[:, :180] = re[:, :180] + re[:, 360:180:-1]
    xe_re[:, 180] = re[:, 180]
    xe_im[:, :180] = im[:, :180] + im[:, 360:180:-1]
    xe_im[:, 180] = im[:, 180]
    xo_re[:, :180] = re[:, :180] - re[:, 360:180:-1]
    xo_im[:, :180] = im[:, :180] - im[:, 360:180:-1]

    wtf = weights.transpose(0, 2, 1)  # (m, k, l)
    in_maps = []
    for j in range(NCORES):
        xw = np.zeros((MPC, 256, 1388), dtype=bf)
        e_first = j % 2 == 0  # pass A symmetric for even cores
        for i in range(MPC):
            m = NCORES * i + j
            if m >= MMAX:
                continue
            klo, hw = windows[i]
            khi = klo + hw
            nlA, nlB = _nlab(i)
            lA = np.arange(NCORES * i, LMAX, 2)
            lB = np.arange(NCORES * i + 1, LMAX, 2)
            s0r, s0i = (xe_re, xe_im) if e_first else (xo_re, xo_im)
            s1r, s1i = (xo_re, xo_im) if e_first else (xe_re, xe_im)
            xw[i, :hw, 0:256] = s0r[:, klo:khi, m].T
            xw[i, :hw, 256:512] = s0i[:, klo:khi, m].T
            xw[i, :hw, 512:768] = s1r[:, klo:khi, m].T
            xw[i, :hw, 768:1024] = s1i[:, klo:khi, m].T
            xw[i, :hw, 1024 : 1024 + nlA] = wtf[m, klo:khi][:, lA]
            xw[i, :hw, 1024 + nlA : 1024 + nlA + nlB] = wtf[m, klo:khi][:, lB]
        in_maps.append({"xw": xw})
    return in_maps


def unpack_stage_b(results):
    out = np.zeros((1, C, LMAX, MMAX), dtype=np.complex64)
    for j in range(NCORES):
        bo = np.asarray(results[j]["bout"], dtype=np.float32)  # (MPC,2,128,1024)
        for i in range(MPC):
            m = NCORES * i + j
            if m >= MMAX:
                continue
            nlA, nlB = _nlab(i)
            lA = np.arange(NCORES * i, LMAX, 2)
            lB = np.arange(NCORES * i + 1, LMAX, 2)
            for tp in range(-(-nlA // 128)):
                lpA = min(128, nlA - tp * 128)
                lpB = min(128, nlB - tp * 128)
                blk = bo[i, tp]
                sl = slice(tp * 128, tp * 128 + lpA)
                out[0][:, lA[sl], m] = (blk[:lpA, :256] + 1j * blk[:lpA, 256:512]).T
                if lpB > 0:
                    slB = slice(tp * 128, tp * 128 + lpB)
                    out[0][:, lB[slB], m] = (
                        blk[:lpB, 512:768] + 1j * blk[:lpB, 768:1024]
                    ).T
    return out


# ------------------------------------------------------------------ driver ---


def _install_ntff_hook():
    """This image's antenv lacks axon_hooks; synthesize it so bass_utils'
    trace=True path can capture NTFFs via the axon PJRT .so."""
    import sys

    if "antenv.axon_hooks" in sys.modules:
        return
    import types

    mod = types.ModuleType("antenv.axon_hooks")
    state = {"hook": None}
    mod.set_axon_ntff_profile_hook = lambda h: state.__setitem__("hook", h)
    mod.get_axon_ntff_profile_hook = lambda: state["hook"]
    sys.modules["antenv.axon_hooks"] = mod
    try:
        import importlib.util as ilu

        spec = ilu.spec_from_file_location(
            "_trn_boot_hook", "/root/.axon_site/trn_agent_boot/trn_boot.py"
        )
        tb = ilu.module_from_spec(spec)
        spec.loader.exec_module(tb)
        mod.set_axon_ntff_profile_hook(
            tb._ntff_profile_via_ctypes("/opt/axon/libaxon_pjrt.so")
        )
    except Exception:
        pass


def _run(nc, in_maps, label):
    kw = {}
    if os.environ.get("SHT_TRACE"):
        import concourse.bass_utils as bu

        bu.upload_artifacts = lambda tmpdir: tmpdir  # no S3 in this sandbox
        _install_ntff_hook()
        kw = dict(trace=True)
    try:
        res = run_bass_kernel_spmd(nc, in_maps, core_ids=list(range(NCORES)), **kw)
    except Exception:
        if not kw:
            raise
        res = run_bass_kernel_spmd(nc, in_maps, core_ids=list(range(NCORES)))
    LAST_PERF[label] = res.exec_time_ns
    return res


def kernel(x, weights):
    x = np.asarray(x, dtype=np.float32).reshape(C, NLAT, NLON)
    weights = np.asarray(weights, dtype=np.float32)

    windows = compute_windows(weights)
    klo2 = windows[32][0]  # window of the m>=256 group block
    kw2 = NLAT - 2 * klo2  # unfolded window width for m >= 256
    kw2e = kw2 + (kw2 & 1)

    xin, mats = pack_stage_a_inputs(x)
    nc_a = build_stage_a(klo2, kw2e)
    in_maps = [
        {"xin": xin[j * CPC : (j + 1) * CPC], "mats": mats} for j in range(NCORES)
    ]
    res_a = _run(nc_a, in_maps, "stage_a")
    re, im = unpack_stage_a(res_a.results, klo2, kw2)

    in_maps_b = pack_stage_b_inputs(re, im, weights, windows)
    nc_b = build_stage_b([hw for _, hw in windows])
    res_b = _run(nc_b, in_maps_b, "stage_b")
    return unpack_stage_b(res_b.results)


# revision 7
# speedup vs baseline: 1.1693x; 1.1693x over previous
"""Distributed real SHT (spherical harmonic transform) on 8 trn2 NeuronCores.

Pipeline:
  out[b,c,l,m] = sum_k W[m,l,k] * XF[b,c,m,k],   XF = (2*pi/nlon) * rfft(x, lon)[..., :mmax]

Stage A (launch 1, channel-sharded): DFT along longitude as bf16 matmuls with a
DOUBLE longitude fold: n -> n' (rfft realness, cos/sin split) and n' -> n''
(reflection about nlon/4) which splits m by parity. Four quadrant DFTs
(cos-even, cos-odd, sin-even, sin-odd) of ~181x181 replace two 361x361 ones.
The DFT matrices are the STATIONARY operand (psum partitions = m) so the
moving operand streams 362 latitude columns per matmul -- long enough to hide
LDWEIGHTS, which a short-stream layout exposes (~148ns/matmul floor). The
m>=256 half-tiles stream only the latitude window where any P_l^m (m>=256) is
non-negligible.

Host exchange (free): unfold quadrants to XF[c,k,m], fold latitude using
P_l^m(pi-th) = (-1)^(l+m) P_l^m(th)  ->  xe/xo parts on k' in [0,181).

Stage B (launch 2, m-sharded, m interleaved mod 8): folded Legendre contraction
  psum[l_tile, (ri,c)=512] += W[m, l_par, k'_chunk]^T @ x_par[k'_chunk, 512]
over the exact per-group latitude window [klo, 181). Both l-parities
(even/odd l-m -> xe/xo) are computed per m into one 2-bank psum pair tile so a
single wide cast drains both. rhs and weights live in one dram tensor but load
into SEPARATE tiles (weights-as-stationary needs its own tile to get the
double-buffered LDWEIGHTS path; same-tile operands fall into a ~250ns/matmul
slower fused path).

DMA packet cost is ~9ns + bytes/(20-27GB/s) per partition-line across 16
engines, so every DRAM layout here is packed for >=1-3KB contiguous lines.
bf16 operands; psum accumulation is fp32. fp8 was measured to break the 2e-2
error budget (2.7e-2 with fp8 XF alone), so everything stays bf16.
"""

import os

import numpy as np

import concourse.bacc as bacc
import concourse.mybir as mybir
from concourse.tile import TileContext
from concourse.bass_utils import run_bass_kernel_spmd

LAST_PERF = {}

NLAT = 361
NLON = 720
MMAX = 361
LMAX = 361
C = 256
NCORES = 8
CPC = C // NCORES  # 32 channels per core
MPC = (MMAX + NCORES - 1) // NCORES  # 46 m's per core (padded)

MH = 182  # m-half columns, padded even (ce:181, co:180, se:181, so:180)
KCOLS = 362  # nlat padded even
NQR = [181, 180, 179, 180]  # quadrant fold-row counts (ce, co, se, so)
NP2 = [q - 128 for q in NQR]  # partial contraction-chunk rows [53, 52, 51, 52]
MW1 = [53, 52, 53, 52]  # m-cols of the second m-tile per quadrant
KHALF = 181  # folded latitude rows

F32 = mybir.dt.float32
BF16 = mybir.dt.bfloat16


# ---------------------------------------------------------------- stage A ----


def build_stage_a(klo2, kw2e, cpc=CPC):
    """xin [cpc, 181, 4*KCOLS] bf16: rows 0:128 = contraction chunk0 of each
    quadrant, rows 128:181 = partial chunk1 (quadrant q at cols q*KCOLS, rows
    beyond the quadrant's real count are host-zeroed).  mats [181, 4*MH]: same
    row split; quadrant q's DFT matrix columns (m-half) at q*MH.
    aoutA [cpc, 128, 4*KCOLS]: m-tile0 (m-half rows 0:128) x full k.
    aoutB [cpc, 53, 4*kw2e]: m-tile1 (m-half rows 128:) x k in [klo2, +kw2e).

    Matmuls are batched into uniform-shape phases (constant contraction rows
    and constant moving tile within a phase): the PE pays a ~100-300ns
    reconfiguration penalty whenever consecutive matmuls change contraction
    row count or moving-operand tile, so the tail chunks (53 zero-padded rows)
    run as one phase and the head chunks (128 rows) as another, accumulating
    into the same psums."""
    nc = bacc.Bacc("TRN2", target_bir_lowering=False)
    xin = nc.dram_tensor("xin", [cpc, 181, 4 * KCOLS], BF16, kind="ExternalInput")
    mats = nc.dram_tensor("mats", [181, 4 * MH], BF16, kind="ExternalInput")
    aoutA = nc.dram_tensor("aoutA", [cpc, 128, 4 * KCOLS], BF16, kind="ExternalOutput")
    aoutB = nc.dram_tensor("aoutB", [cpc, 53, 4 * kw2e], BF16, kind="ExternalOutput")

    cast_idx = 0
    with TileContext(nc) as tc:
        with (
            tc.tile_pool(name="mats", bufs=1) as matp,
            tc.tile_pool(name="xinp", bufs=3) as xinp,
            tc.tile_pool(name="outp", bufs=3) as outp,
            tc.tile_pool(name="ps", bufs=4, space="PSUM") as psp,
        ):
            mt1 = matp.tile([128, 4 * MH], BF16, tag="m1")
            mt2 = matp.tile([64, 4 * MH], BF16, tag="m2")
            nc.sync.dma_start(out=mt1, in_=mats[:128, :])
            nc.sync.dma_start(out=mt2[:53], in_=mats[128:181, :])
            for c in range(cpc):
                xt1 = xinp.tile([128, 4 * KCOLS], BF16, tag="x1")
                xt2 = xinp.tile([64, 4 * KCOLS], BF16, tag="x2")
                nc.sync.dma_start(out=xt1, in_=xin[c, :128, :])
                nc.sync.dma_start(out=xt2[:53], in_=xin[c, 128:181, :])
                otA = outp.tile([128, 4 * KCOLS], BF16, tag="oA")
                otB = outp.tile([53, 4 * kw2e], BF16, tag="oB")
                ps0 = [psp.tile([128, KCOLS], F32, tag="ps0", name=f"ps0_{c}_{q}") for q in range(4)]
                ps1 = [psp.tile([53, kw2e], F32, tag="ps1", name=f"ps1_{c}_{q}") for q in range(4)]
                # phase T0: tails of m-tile0 (rows 53, xt2, free 362)
                for q in range(4):
                    nc.tensor.matmul(
                        ps0[q][:, :],
                        mt2[:53, q * MH : q * MH + 128],
                        xt2[:53, q * KCOLS : q * KCOLS + KCOLS],
                        start=True,
                        stop=False,
                    )
                # phase T1: tails of m-tile1 (rows 53, xt2, free kw2e)
                for q in range(4):
                    nc.tensor.matmul(
                        ps1[q][: MW1[q], :],
                        mt2[:53, q * MH + 128 : q * MH + 128 + MW1[q]],
                        xt2[:53, q * KCOLS + klo2 : q * KCOLS + klo2 + kw2e],
                        start=True,
                        stop=False,
                    )
                # phase H0: heads of m-tile0 (rows 128, xt1, free 362)
                for q in range(4):
                    nc.tensor.matmul(
                        ps0[q][:, :],
                        mt1[:, q * MH : q * MH + 128],
                        xt1[:, q * KCOLS : q * KCOLS + KCOLS],
                        start=False,
                        stop=True,
                    )
                # phase H1: heads of m-tile1 (rows 128, xt1, free kw2e)
                for q in range(4):
                    nc.tensor.matmul(
                        ps1[q][: MW1[q], :],
                        mt1[:, q * MH + 128 : q * MH + 128 + MW1[q]],
                        xt1[:, q * KCOLS + klo2 : q * KCOLS + klo2 + kw2e],
                        start=False,
                        stop=True,
                    )
                for q in range(4):
                    for dst, src in (
                        (otA[:, q * KCOLS : (q + 1) * KCOLS], ps0[q][:, :]),
                        (otB[: MW1[q], q * kw2e : q * kw2e + kw2e], ps1[q][: MW1[q], :]),
                    ):
                        if cast_idx % 2 == 0:
                            nc.vector.tensor_copy(out=dst, in_=src)
                        else:
                            nc.scalar.copy(dst, src)
                        cast_idx += 1
                nc.gpsimd.dma_start(out=aoutA[c], in_=otA)
                nc.gpsimd.dma_start(out=aoutB[c], in_=otB)
    nc.compile()
    return nc


def _quadrant_mats():
    """DFT quadrant matrices [n'' rows, m-half cols], 2*pi/nlon scale folded.
    Verified exact against np.fft.rfft."""
    s = 2.0 * np.pi / NLON
    m = np.arange(MMAX)
    npp = np.arange(181)
    ang = 2.0 * np.pi * np.outer(npp, m % NLON) / NLON
    cosm = s * np.cos(ang)  # [n''=0..180, m]
    sinm = -s * np.sin(ang)
    mat_ce = cosm[:181][:, m % 2 == 0]  # 181 x 181
    mat_co = cosm[:180][:, m % 2 == 1]  # 180 x 180
    mat_se = sinm[1:180][:, m % 2 == 0]  # 179 x 181
    mat_so = sinm[1:181][:, m % 2 == 1]  # 180 x 180
    return [mat_ce, mat_co, mat_se, mat_so]


def fold_quadrants(x):
    """x: (C, nlat, nlon) f32 -> 4 arrays (C, nlat, rows_q): folded inputs for
    the quadrant DFTs (ce, co, se, so)."""
    xc = np.empty((x.shape[0], x.shape[1], 361), dtype=np.float32)
    xc[..., 0] = x[..., 0]
    xc[..., 360] = x[..., 360]
    xc[..., 1:360] = x[..., 1:360] + x[..., :360:-1]
    xsf = np.zeros((x.shape[0], x.shape[1], 361), dtype=np.float32)
    xsf[..., 1:360] = x[..., 1:360] - x[..., :360:-1]

    xce = np.empty((x.shape[0], x.shape[1], 181), dtype=np.float32)
    xce[..., :180] = xc[..., :180] + xc[..., 360:180:-1]
    xce[..., 180] = xc[..., 180]
    xco = xc[..., :180] - xc[..., 360:180:-1]
    xse = xsf[..., 1:180] - xsf[..., 359:180:-1]
    xso = np.empty((x.shape[0], x.shape[1], 180), dtype=np.float32)
    xso[..., :179] = xsf[..., 1:180] + xsf[..., 359:180:-1]
    xso[..., 179] = xsf[..., 180]
    return [xce, xco, xse, xso]


def pack_stage_a_inputs(x):
    """x: (C, nlat, nlon) f32 -> xin (C, 181, 4*KCOLS) bf16, mats (181, 4*MH)
    bf16."""
    import ml_dtypes

    bf = ml_dtypes.bfloat16
    quads = fold_quadrants(x)
    xin = np.zeros((x.shape[0], 181, 4 * KCOLS), dtype=bf)
    for q, xq in enumerate(quads):
        nq = NQR[q]
        xt = xq.transpose(0, 2, 1)  # (C, rows, k)
        xin[:, :128, q * KCOLS : q * KCOLS + NLAT] = xt[:, :128].astype(bf)
        xin[:, 128 : 128 + NP2[q], q * KCOLS : q * KCOLS + NLAT] = xt[:, 128:nq].astype(
            bf
        )
    mats = np.zeros((181, 4 * MH), dtype=bf)
    for q, mq in enumerate(_quadrant_mats()):
        nq, ncol = mq.shape
        mats[:128, q * MH : q * MH + ncol] = mq[:128].astype(bf)
        mats[128 : 128 + NP2[q], q * MH : q * MH + ncol] = mq[128:nq].astype(bf)
    return xin, mats


def unpack_stage_a(results, klo2, kw2):
    """-> re, im arrays (C, nlat, mmax) f32.  XF for m>=256 outside the
    [klo2, klo2+kw2) latitude window is left zero (stage B never reads it)."""
    arrA = np.concatenate(
        [np.asarray(r["aoutA"], dtype=np.float32) for r in results], axis=0
    ).reshape(C, 128, 4, KCOLS)
    arrB = np.concatenate(
        [np.asarray(r["aoutB"], dtype=np.float32) for r in results], axis=0
    )
    kw2e = arrB.shape[-1] // 4
    arrB = arrB.reshape(C, 53, 4, kw2e)
    re = np.zeros((C, NLAT, MMAX), dtype=np.float32)
    im = np.zeros((C, NLAT, MMAX), dtype=np.float32)
    # m-tile 0: m-half rows 0..127 -> m = 2r (+1), all k
    re[:, :, 0:256:2] = arrA[:, :, 0, :NLAT].transpose(0, 2, 1)
    re[:, :, 1:256:2] = arrA[:, :, 1, :NLAT].transpose(0, 2, 1)
    im[:, :, 0:256:2] = arrA[:, :, 2, :NLAT].transpose(0, 2, 1)
    im[:, :, 1:256:2] = arrA[:, :, 3, :NLAT].transpose(0, 2, 1)
    # m-tile 1: m-half rows 128.. -> m = 256+2r (+1), k in [klo2, klo2+kw2)
    ksl = slice(klo2, klo2 + kw2)
    re[:, ksl, 256::2] = arrB[:, :53, 0, :kw2].transpose(0, 2, 1)
    re[:, ksl, 257::2] = arrB[:, :52, 1, :kw2].transpose(0, 2, 1)
    im[:, ksl, 256::2] = arrB[:, :53, 2, :kw2].transpose(0, 2, 1)
    im[:, ksl, 257::2] = arrB[:, :52, 3, :kw2].transpose(0, 2, 1)
    return re, im


# ---------------------------------------------------------------- stage B ----


def _nlab(i):
    nl = LMAX - NCORES * i
    return (nl + 1) // 2, nl // 2


def b_blocks(hw_list):
    """Group the m-indices into blocks of <=2 with equal chunk count, heavy
    and light interleaved within each class; lightest block last."""
    nkc2 = [i for i in range(MPC) if hw_list[i] > 128]
    nkc1 = [i for i in range(MPC) if hw_list[i] <= 128]
    blocks = []
    for cls in (nkc2, nkc1):
        lo, hi = 0, len(cls) - 1
        while lo < hi:
            blocks.append([cls[lo], cls[hi]])
            lo += 1
            hi -= 1
        if lo == hi:
            blocks.append([cls[lo]])
    # lightest (largest i) singleton last for a minimal drain tail
    blocks.sort(key=lambda b: max(b) == MPC - 1)
    return blocks


def build_stage_b(hw_list):
    """xw [MPC, 256, 1388] bf16: rows = folded-latitude window rows (k'-klo,
    zero beyond the window), cols 0:512 = rhs slot0 (re|im x 256ch),
    512:1024 = slot1, 1024:1388 = weight l-columns [pass-A (nlA) | pass-B
    (nlB)].  bout [MPC, 2, 128, 1024] bf16: [i, l-tile, l-row, A(512)|B(512)].

    Contractions are padded to uniform 128-row chunk0 / 64-row chunk1 (dram
    rows beyond the window are zero) and matmuls are batched per 2-index
    block into a constant-shape chunk0 phase then a chunk1 phase -- changing
    contraction rows or the moving tile between consecutive matmuls costs
    ~100-300ns of PE reconfiguration."""
    nc = bacc.Bacc("TRN2", target_bir_lowering=False)
    xw = nc.dram_tensor("xw", [MPC, 256, 1388], BF16, kind="ExternalInput")
    bout = nc.dram_tensor("bout", [MPC, 2, 128, 1024], BF16, kind="ExternalOutput")

    cast_idx = 0
    with TileContext(nc) as tc:
        with (
            tc.tile_pool(name="rh0", bufs=4) as rp0,
            tc.tile_pool(name="rh1", bufs=4) as rp1,
            tc.tile_pool(name="wt0", bufs=4) as wp0,
            tc.tile_pool(name="wt1", bufs=4) as wp1,
            tc.tile_pool(name="outp", bufs=4) as outp,
            tc.tile_pool(name="ps", bufs=4, space="PSUM") as psp,
        ):
            for blk in b_blocks(hw_list):
                nkc = 2 if hw_list[blk[0]] > 128 else 1
                tiles = {}
                for i in blk:
                    r0 = rp0.tile([128, 1024], BF16, tag="r0")
                    w0 = wp0.tile([128, 364], BF16, tag="w0")
                    nl = LMAX - NCORES * i
                    nc.sync.dma_start(out=r0, in_=xw[i, :128, :1024])
                    nc.scalar.dma_start(out=w0[:, :nl], in_=xw[i, :128, 1024 : 1024 + nl])
                    tiles[i] = [(r0, w0, 128)]
                    if nkc == 2:
                        r1 = rp1.tile([64, 1024], BF16, tag="r1")
                        w1 = wp1.tile([64, 364], BF16, tag="w1")
                        nc.sync.dma_start(out=r1, in_=xw[i, 128:192, :1024])
                        nc.scalar.dma_start(
                            out=w1[:, :nl], in_=xw[i, 128:192, 1024 : 1024 + nl]
                        )
                        tiles[i].append((r1, w1, 64))
                ps = {}
                ot = {}
                for i in blk:
                    nlA, _ = _nlab(i)
                    for tp in range(-(-nlA // 128)):
                        ps[(i, tp)] = psp.tile([128, 1024], F32, tag="ps", name=f"ps_{i}_{tp}")
                        ot[(i, tp)] = outp.tile([128, 1024], BF16, tag="ot", name=f"ot_{i}_{tp}")
                # uniform-shape matmul phases: chunk0 (128 rows), chunk1 (64)
                for ck in range(nkc):
                    for i in blk:
                        nlA, nlB = _nlab(i)
                        rt, wt, rows = tiles[i][ck]
                        for tp in range(-(-nlA // 128)):
                            for s, (nls, coff) in enumerate([(nlA, 0), (nlB, nlA)]):
                                lp = min(128, nls - tp * 128)
                                if lp <= 0:
                                    continue
                                nc.tensor.matmul(
                                    ps[(i, tp)][:lp, s * 512 : (s + 1) * 512],
                                    wt[:rows, coff + tp * 128 : coff + tp * 128 + lp],
                                    rt[:rows, s * 512 : (s + 1) * 512],
                                    start=(ck == 0),
                                    stop=(ck == nkc - 1),
                                )
                for i in blk:
                    nlA, nlB = _nlab(i)
                    for tp in range(-(-nlA // 128)):
                        rows = min(128, nlA - tp * 128)
                        # one wide cast drains both parity passes; gpsimd
                        # cannot read PSUM, so split vector-heavy/scalar-light
                        if cast_idx % 5 == 4:
                            nc.scalar.copy(ot[(i, tp)][:rows, :], ps[(i, tp)][:rows, :])
                        else:
                            nc.vector.tensor_copy(
                                out=ot[(i, tp)][:rows, :], in_=ps[(i, tp)][:rows, :]
                            )
                        cast_idx += 1
                        nc.gpsimd.dma_start(
                            out=bout[i, tp, :rows, :], in_=ot[(i, tp)][:rows, :]
                        )
    nc.compile()
    return nc


def compute_windows(weights):
    """Folded-latitude window per m-group: klo_i = first k' (0..180) where any
    |W[m,l,k']| with m in group i is non-negligible; support always reaches the
    equator k'=180."""
    wabs = np.abs(weights).max(axis=1)  # (m, k)
    thr = 1e-7 * wabs.max()
    klo_m = np.empty(MMAX, dtype=np.int64)
    for m in range(MMAX):
        nz = np.nonzero(wabs[m, :KHALF] > thr)[0]
        klo_m[m] = nz[0] if len(nz) else KHALF - 1
    windows = []
    for i in range(MPC):
        ms = [NCORES * i + j for j in range(NCORES) if NCORES * i + j < MMAX]
        klo = int(min(klo_m[m] for m in ms))
        windows.append((klo, KHALF - klo))
    return windows


def pack_stage_b_inputs(re, im, weights, windows):
    """re/im: (C, nlat, mmax) f32.  -> per-core xw tensors (bf16)."""
    import ml_dtypes

    bf = ml_dtypes.bfloat16
    # latitude fold (host, f32)
    xe_re = np.empty((C, KHALF, MMAX), dtype=np.float32)
    xe_im = np.empty((C, KHALF, MMAX), dtype=np.float32)
    xo_re = np.zeros((C, KHALF, MMAX), dtype=np.float32)
    xo_im = np.zeros((C, KHALF, MMAX), dtype=np.float32)
    xe_re[:, :180] = re[:, :180] + re[:, 360:180:-1]
    xe_re[:, 180] = re[:, 180]
    xe_im[:, :180] = im[:, :180] + im[:, 360:180:-1]
    xe_im[:, 180] = im[:, 180]
    xo_re[:, :180] = re[:, :180] - re[:, 360:180:-1]
    xo_im[:, :180] = im[:, :180] - im[:, 360:180:-1]

    wtf = weights.transpose(0, 2, 1)  # (m, k, l)
    in_maps = []
    for j in range(NCORES):
        xw = np.zeros((MPC, 256, 1388), dtype=bf)
        e_first = j % 2 == 0  # pass A symmetric for even cores
        for i in range(MPC):
            m = NCORES * i + j
            if m >= MMAX:
                continue
            klo, hw = windows[i]
            khi = klo + hw
            nlA, nlB = _nlab(i)
            lA = np.arange(NCORES * i, LMAX, 2)
            lB = np.arange(NCORES * i + 1, LMAX, 2)
            s0r, s0i = (xe_re, xe_im) if e_first else (xo_re, xo_im)
            s1r, s1i = (xo_re, xo_im) if e_first else (xe_re, xe_im)
            xw[i, :hw, 0:256] = s0r[:, klo:khi, m].T
            xw[i, :hw, 256:512] = s0i[:, klo:khi, m].T
            xw[i, :hw, 512:768] = s1r[:, klo:khi, m].T
            xw[i, :hw, 768:1024] = s1i[:, klo:khi, m].T
            xw[i, :hw, 1024 : 1024 + nlA] = wtf[m, klo:khi][:, lA]
            xw[i, :hw, 1024 + nlA : 1024 + nlA + nlB] = wtf[m, klo:khi][:, lB]
        in_maps.append({"xw": xw})
    return in_maps


def unpack_stage_b(results):
    out = np.zeros((1, C, LMAX, MMAX), dtype=np.complex64)
    for j in range(NCORES):
        bo = np.asarray(results[j]["bout"], dtype=np.float32)  # (MPC,2,128,1024)
        for i in range(MPC):
            m = NCORES * i + j
            if m >= MMAX:
                continue
            nlA, nlB = _nlab(i)
            lA = np.arange(NCORES * i, LMAX, 2)
            lB = np.arange(NCORES * i + 1, LMAX, 2)
            for tp in range(-(-nlA // 128)):
                lpA = min(128, nlA - tp * 128)
                lpB = min(128, nlB - tp * 128)
                blk = bo[i, tp]
                sl = slice(tp * 128, tp * 128 + lpA)
                out[0][:, lA[sl], m] = (blk[:lpA, :256] + 1j * blk[:lpA, 256:512]).T
                if lpB > 0:
                    slB = slice(tp * 128, tp * 128 + lpB)
                    out[0][:, lB[slB], m] = (
                        blk[:lpB, 512:768] + 1j * blk[:lpB, 768:1024]
                    ).T
    return out


# ------------------------------------------------------------------ driver ---


def _install_ntff_hook():
    """This image's antenv lacks axon_hooks; synthesize it so bass_utils'
    trace=True path can capture NTFFs via the axon PJRT .so."""
    import sys

    if "antenv.axon_hooks" in sys.modules:
        return
    import types

    mod = types.ModuleType("antenv.axon_hooks")
    state = {"hook": None}
    mod.set_axon_ntff_profile_hook = lambda h: state.__setitem__("hook", h)
    mod.get_axon_ntff_profile_hook = lambda: state["hook"]
    sys.modules["antenv.axon_hooks"] = mod
    try:
        import importlib.util as ilu

        spec = ilu.spec_from_file_location(
            "_trn_boot_hook", "/root/.axon_site/trn_agent_boot/trn_boot.py"
        )
        tb = ilu.module_from_spec(spec)
        spec.loader.exec_module(tb)
        mod.set_axon_ntff_profile_hook(
            tb._ntff_profile_via_ctypes("/opt/axon/libaxon_pjrt.so")
        )
    except Exception:
        pass


def _run(nc, in_maps, label):
    kw = {}
    if os.environ.get("SHT_TRACE"):
        import concourse.bass_utils as bu

        bu.upload_artifacts = lambda tmpdir: tmpdir  # no S3 in this sandbox
        _install_ntff_hook()
        kw = dict(trace=True)
    try:
        res = run_bass_kernel_spmd(nc, in_maps, core_ids=list(range(NCORES)), **kw)
    except Exception:
        if not kw:
            raise
        res = run_bass_kernel_spmd(nc, in_maps, core_ids=list(range(NCORES)))
    LAST_PERF[label] = res.exec_time_ns
    return res


def kernel(x, weights):
    x = np.asarray(x, dtype=np.float32).reshape(C, NLAT, NLON)
    weights = np.asarray(weights, dtype=np.float32)

    windows = compute_windows(weights)
    klo2 = windows[32][0]  # window of the m>=256 group block
    kw2 = NLAT - 2 * klo2  # unfolded window width for m >= 256
    kw2e = kw2 + (kw2 & 1)

    xin, mats = pack_stage_a_inputs(x)
    nc_a = build_stage_a(klo2, kw2e)
    in_maps = [
        {"xin": xin[j * CPC : (j + 1) * CPC], "mats": mats} for j in range(NCORES)
    ]
    res_a = _run(nc_a, in_maps, "stage_a")
    re, im = unpack_stage_a(res_a.results, klo2, kw2)

    in_maps_b = pack_stage_b_inputs(re, im, weights, windows)
    nc_b = build_stage_b([hw for _, hw in windows])
    res_b = _run(nc_b, in_maps_b, "stage_b")
    return unpack_stage_b(res_b.results)


# revision 10
# speedup vs baseline: 2.0471x; 1.7508x over previous
"""Distributed real SHT (spherical harmonic transform) on 8 trn2 NeuronCores.

Pipeline:
  out[b,c,l,m] = sum_k W[m,l,k] * XF[b,c,m,k],   XF = (2*pi/nlon) * rfft(x, lon)[..., :mmax]

Stage A (launch 1, channel-sharded): DFT along longitude as bf16 matmuls.
  Host folds x over lon parity (cos: n'=0..360, sin: n'=1..359), transposes to
  [c, n', k] (contraction n' on partitions), pads n' chunks to 3x128 per parity
  and k to 362, packs cos+sin into one tensor so each channel loads with ONE DMA.
  psum[k_tile, m] += xT[n'chunk, k_tile]^T @ DFTmat[n'chunk, m]
Host exchange: XF[c,k,m] (channel-sharded) -> XFB[m_local, k, (ri,c)] (m-sharded).
Stage B (launch 2, m-sharded, m interleaved mod 8 for triangular balance):
  psum[l_tile, 512] += WT[m][k, l_tile]^T @ XFB[m][k, (ri,c)=512]
  Only l >= m is computed (weights are exactly zero below the diagonal).

bf16 operands keep the PE at 2.4 GHz (fp32r matmuls don't count as HAM activity
and pin the array at 1.2 GHz) and halve DMA bytes; psum accumulation is fp32.
k is padded to 384=3x128 so rhs/weight loads are one rearranged DMA per tile.
"""

import os

import numpy as np

import concourse.bacc as bacc
import concourse.mybir as mybir
from concourse.tile import TileContext
from concourse.bass_utils import run_bass_kernel_spmd

LAST_PERF = {}

NLAT = 361
NLON = 720
MMAX = 361
LMAX = 361
C = 256
NCORES = 8
CPC = C // NCORES  # 32 channels per core
NC_COS = NLON // 2 + 1  # 361 cos columns (n'=0..360)
NC_SIN = NLON // 2 - 1  # 359 sin columns (n'=1..359)
MPC = (MMAX + NCORES - 1) // NCORES  # 46 m's per core (padded)
KPAD = 384  # nlat padded to 3x128 partition chunks
NPAD = 384  # per-parity n' padded to 3x128
MEVEN = 362  # k (stage A moving free dim) padded even

F32 = mybir.dt.float32
BF16 = mybir.dt.bfloat16


def _ptiles(n, p=128):
    out = []
    o = 0
    while o < n:
        out.append((o, min(p, n - o)))
        o += p
    return out


def build_stage_a(cpc=CPC, nlat=NLAT, mmax=MMAX):
    """Inputs: xin [cpc, 2*NPAD, MEVEN] bf16 (cos rows 0:361, sin rows 384:743,
    both zero-padded; last col zero), mats [2*NPAD, MEVEN] bf16 (same row
    packing; DFT matrices with the 2*pi/nlon scale; col dim is m padded even).
    Outputs: xfr/xfi [cpc, KPAD, mmax] bf16 (k rows >= 361 are garbage)."""
    nc = bacc.Bacc("TRN2", target_bir_lowering=False)
    nseg0 = 2 * NPAD // 128
    xin = nc.dram_tensor("xin", [cpc, 128, nseg0 * MEVEN], BF16, kind="ExternalInput")
    mats = nc.dram_tensor("mats", [128, nseg0 * MEVEN], BF16, kind="ExternalInput")
    xfr = nc.dram_tensor("xfr", [cpc, 128, (KPAD // 128) * mmax], BF16, kind="ExternalOutput")
    xfi = nc.dram_tensor("xfi", [cpc, 128, (KPAD // 128) * mmax], BF16, kind="ExternalOutput")

    nseg = 2 * NPAD // 128  # 6 partition segments: 3 cos + 3 sin
    k_tiles = _ptiles(nlat)  # psum partition tiles over k (128,128,105)
    copy_idx = 0
    with TileContext(nc) as tc:
        with (
            tc.tile_pool(name="mats", bufs=1) as matp,
            tc.tile_pool(name="xinp", bufs=3) as xinp,
            tc.tile_pool(name="outp", bufs=4) as outp,
            tc.tile_pool(name="ps", bufs=6, space="PSUM") as psp,
        ):
            mat_t = matp.tile([128, nseg * MEVEN], BF16, tag="mats")
            nc.sync.dma_start(out=mat_t, in_=mats[:, :])

            for c in range(cpc):
                x_t = xinp.tile([128, nseg * MEVEN], BF16, tag="xin")
                nc.sync.dma_start(out=x_t, in_=xin[c])
                for ri, odram in enumerate((xfr, xfi)):
                    ot = outp.tile([128, len(k_tiles) * mmax], BF16, tag="ot")
                    kp_last = k_tiles[-1][1]
                    # slots are reused round-robin; after the first 4 fills the
                    # pad rows hold stale-but-defined data the host ignores
                    if kp_last < 128 and c < 2:
                        base = (kp_last // 32) * 32  # partition offsets must be 32-aligned
                        nc.gpsimd.memset(
                            ot[base:, (len(k_tiles) - 1) * mmax :], 0.0
                        )
                    for kt, (k0, kp) in enumerate(k_tiles):
                        ps = psp.tile([128, mmax + 1], F32, tag="ps")
                        for s in range(3):
                            seg = 3 * ri + s
                            nc.tensor.matmul(
                                ps[:kp, :],
                                x_t[:, seg * MEVEN + k0 : seg * MEVEN + k0 + kp],
                                mat_t[:, seg * MEVEN : seg * MEVEN + mmax + 1],
                                start=(s == 0),
                                stop=(s == 2),
                            )
                        dst = ot[:kp, kt * mmax : (kt + 1) * mmax]
                        if copy_idx % 2 == 0:
                            nc.vector.tensor_copy(out=dst, in_=ps[:kp, :mmax])
                        else:
                            nc.scalar.copy(dst, ps[:kp, :mmax])
                        copy_idx += 1
                    st_eng = nc.gpsimd if (2 * c + ri) % 2 == 0 else nc.sync
                    st_eng.dma_start(out=odram[c], in_=ot)
    nc.compile()
    return nc


def build_stage_b(mpc=MPC, nlat=NLAT, lmax=LMAX, ncores=NCORES, nkc_list=None):
    """Inputs: xfb [mpc, KPAD, 512] bf16, wt [mpc, KPAD, lmax] bf16 -> out
    [mpc, lmax, 512] bf16. Index i handles m = ncores*i + core_j; computes l in
    [ncores*i, lmax) uniformly across cores (weights are zero for l < m ->
    exact zeros). nkc_list[i] gives the number of 128-row k chunks actually
    contracted for index i (the host packs only the latitude window where
    P_l^m is non-negligible -- it shrinks toward the equator as m grows)."""
    nc = bacc.Bacc("TRN2", target_bir_lowering=False)
    nric = 2 * C
    xfb = nc.dram_tensor("xfb", [mpc, 128, (KPAD // 128) * nric], BF16, kind="ExternalInput")
    wt = nc.dram_tensor("wt", [mpc, 128, (KPAD // 128) * lmax], BF16, kind="ExternalInput")
    # bf16 output halves the store wire; the host upcasts when assembling
    out = nc.dram_tensor("out", [mpc, lmax, nric], BF16, kind="ExternalOutput")

    nkc_max = KPAD // 128
    if nkc_list is None:
        nkc_list = [nkc_max] * mpc
    order = b_order(mpc)
    with TileContext(nc) as tc:
        with (
            tc.tile_pool(name="rhs", bufs=10) as rhsp,
            tc.tile_pool(name="wts", bufs=10) as wtp,
            tc.tile_pool(name="outp", bufs=10) as outp,
            tc.tile_pool(name="ps", bufs=7, space="PSUM") as psp,
        ):
            for bi in range(mpc):
                i = order[bi]  # buffer bi holds data for logical index i
                nkc = nkc_list[i]
                rhs_t = rhsp.tile([128, nkc_max * nric], BF16, tag="rhs")
                eng_a = nc.sync if bi % 2 == 0 else nc.scalar
                eng_b = nc.scalar if bi % 2 == 0 else nc.sync
                eng_a.dma_start(
                    out=rhs_t.rearrange("p (t f) -> p t f", t=nkc_max)[:, :nkc],
                    in_=xfb[i].rearrange("p (t f) -> p t f", t=nkc_max)[:, :nkc],
                )
                l_lo = ncores * i
                w_t = wtp.tile([128, nkc_max * lmax], BF16, tag="wt")
                # opposite HWDGE ring from rhs; only the l >= l_lo triangle
                eng_b.dma_start(
                    out=w_t.rearrange("p (t l) -> p t l", t=nkc_max)[:, :nkc, l_lo:],
                    in_=wt[i].rearrange("p (t l) -> p t l", t=nkc_max)[:, :nkc, l_lo:],
                )
                for l0, lp in _ptiles(lmax - l_lo):
                    la = l_lo + l0
                    ps = psp.tile([128, nric], F32, tag="ps")
                    for kc in range(nkc):
                        nc.tensor.matmul(
                            ps[:lp, :],
                            w_t[:, kc * lmax + la : kc * lmax + la + lp],
                            rhs_t[:, kc * nric : (kc + 1) * nric],
                            start=(kc == 0),
                            stop=(kc == nkc - 1),
                        )
                    ot = outp.tile([128, nric], BF16, tag="ot")
                    nc.vector.tensor_copy(out=ot[:lp, :], in_=ps[:lp, :])
                    nc.gpsimd.dma_start(out=out[i, la : la + lp, :], in_=ot[:lp, :])
    nc.compile()
    return nc


def _dft_matrices():
    """cosm[n', m] = s*cos(2 pi m n'/nlon), n'=0..360
    sinm[n', m] = -s*sin(2 pi m n'/nlon), n'=1..359 (imag of rfft = -sum x sin)."""
    s = 2.0 * np.pi / NLON
    m = np.arange(MMAX)
    nc_ = np.arange(NC_COS)
    ns_ = np.arange(1, NLON // 2)
    ang_c = 2.0 * np.pi * ((nc_[:, None] * m[None, :]) % NLON) / NLON
    ang_s = 2.0 * np.pi * ((ns_[:, None] * m[None, :]) % NLON) / NLON
    return (s * np.cos(ang_c)).astype(np.float32), (-s * np.sin(ang_s)).astype(
        np.float32
    )


def fold_x(x):
    """x: (C, nlat, nlon) f32 -> xc (C, nlat, 361), xs (C, nlat, 359)."""
    xc = np.empty((x.shape[0], x.shape[1], NC_COS), dtype=np.float32)
    xc[..., 0] = x[..., 0]
    xc[..., NLON // 2] = x[..., NLON // 2]
    xc[..., 1 : NLON // 2] = x[..., 1 : NLON // 2] + x[..., : NLON // 2 : -1]
    xs = x[..., 1 : NLON // 2] - x[..., : NLON // 2 : -1]
    return xc, np.ascontiguousarray(xs.astype(np.float32))


def pack_stage_a_inputs(x):
    """x: (C, nlat, nlon) f32 -> xin (C, 768, 362) bf16, mats (768, 362) bf16."""
    import ml_dtypes

    bf = ml_dtypes.bfloat16
    xc, xs = fold_x(x)
    xin = np.zeros((x.shape[0], 2 * NPAD, MEVEN), dtype=bf)
    xin[:, :NC_COS, :NLAT] = xc.transpose(0, 2, 1).astype(bf)
    xin[:, NPAD : NPAD + NC_SIN, :NLAT] = xs.transpose(0, 2, 1).astype(bf)
    cosm, sinm = _dft_matrices()
    mats = np.zeros((2 * NPAD, MEVEN), dtype=bf)
    mats[:NC_COS, :MMAX] = cosm.astype(bf)
    mats[NPAD : NPAD + NC_SIN, :MMAX] = sinm.astype(bf)
    # repack rows (s*128+p) -> [p, (s k)] so each channel loads/stores with
    # single DMAs of ~4.3KB contiguous per-partition lines (DMA engines cost
    # ~9ns + bytes/27GB/s per line; 724B lines only reach ~60% of peak)
    nseg = 2 * NPAD // 128
    xin = np.ascontiguousarray(
        xin.reshape(x.shape[0], nseg, 128, MEVEN).transpose(0, 2, 1, 3)
    ).reshape(x.shape[0], 128, nseg * MEVEN)
    mats = np.ascontiguousarray(
        mats.reshape(nseg, 128, MEVEN).transpose(1, 0, 2)
    ).reshape(128, nseg * MEVEN)
    return xin, mats


def b_order(mpc):
    """Interleave heavy (small i, 3 l-tiles) and light (large i) iterations;
    the lightest index runs last so the post-matmul drain tail is minimal."""
    order = []
    lo, hi = 0, mpc - 2
    while lo <= hi:
        order.append(lo)
        if hi != lo:
            order.append(hi)
        lo += 1
        hi -= 1
    order.append(mpc - 1)
    return order


def m_list(j):
    return [NCORES * i + j for i in range(MPC) if NCORES * i + j < MMAX]


def _install_ntff_hook():
    """This image's antenv lacks axon_hooks; synthesize it so bass_utils'
    trace=True path can capture NTFFs via the axon PJRT .so."""
    import sys

    if "antenv.axon_hooks" in sys.modules:
        return
    import types

    mod = types.ModuleType("antenv.axon_hooks")
    state = {"hook": None}
    mod.set_axon_ntff_profile_hook = lambda h: state.__setitem__("hook", h)
    mod.get_axon_ntff_profile_hook = lambda: state["hook"]
    sys.modules["antenv.axon_hooks"] = mod
    try:
        import importlib.util as ilu

        spec = ilu.spec_from_file_location(
            "_trn_boot_hook", "/root/.axon_site/trn_agent_boot/trn_boot.py"
        )
        tb = ilu.module_from_spec(spec)
        spec.loader.exec_module(tb)
        mod.set_axon_ntff_profile_hook(
            tb._ntff_profile_via_ctypes("/opt/axon/libaxon_pjrt.so")
        )
    except Exception:
        pass


def _run(nc, in_maps, label):
    kw = {}
    if os.environ.get("SHT_TRACE"):
        import concourse.bass_utils as bu

        bu.upload_artifacts = lambda tmpdir: tmpdir  # no S3 in this sandbox
        _install_ntff_hook()
        kw = dict(trace=True)
    try:
        res = run_bass_kernel_spmd(nc, in_maps, core_ids=list(range(NCORES)), **kw)
    except Exception:
        if not kw:
            raise
        res = run_bass_kernel_spmd(nc, in_maps, core_ids=list(range(NCORES)))
    LAST_PERF[label] = res.exec_time_ns
    return res


def kernel(x, weights):
    import ml_dtypes

    bf = ml_dtypes.bfloat16
    x = np.asarray(x, dtype=np.float32).reshape(C, NLAT, NLON)
    weights = np.asarray(weights, dtype=np.float32)

    xin, mats = pack_stage_a_inputs(x)
    nc_a = build_stage_a()
    in_maps = [
        {"xin": xin[j * CPC : (j + 1) * CPC], "mats": mats} for j in range(NCORES)
    ]
    res_a = _run(nc_a, in_maps, "stage_a")
    # [c, p, (t m)] -> (C, k=t*128+p, m), drop k padding rows
    nt = KPAD // 128
    xfr = np.concatenate(
        [np.asarray(r["xfr"]).reshape(-1, 128, nt, MMAX) for r in res_a.results], axis=0
    ).transpose(0, 2, 1, 3).reshape(C, KPAD, MMAX)[:, :NLAT, :]
    xfi = np.concatenate(
        [np.asarray(r["xfi"]).reshape(-1, 128, nt, MMAX) for r in res_a.results], axis=0
    ).transpose(0, 2, 1, 3).reshape(C, KPAD, MMAX)[:, :NLAT, :]

    wtf = weights.transpose(0, 2, 1).astype(bf)  # (m, k, l)
    # per-index latitude windows: union of |W| support over the 8 cores' m's
    wabs = np.abs(weights).max(axis=1)  # (m, k)
    thr = 1e-7 * wabs.max()
    windows = []
    for i in range(MPC):
        ms = [NCORES * i + j for j in range(NCORES) if NCORES * i + j < MMAX]
        nz = np.nonzero(wabs[ms].max(axis=0) > thr)[0]
        klo, khi = (int(nz[0]), int(nz[-1]) + 1) if len(nz) else (0, NLAT)
        span = min(-(-max(khi - klo, 1) // 128) * 128, KPAD)
        klo = max(0, min(klo, NLAT - span)) if span < NLAT else 0
        windows.append((klo, span))
    nkc_list = [span // 128 for _, span in windows]
    in_maps_b = []
    for j in range(NCORES):
        ml = m_list(j)
        xfb = np.zeros((MPC, KPAD, 2 * C), dtype=bf)
        wtj = np.zeros((MPC, KPAD, LMAX), dtype=bf)
        for i in range(MPC):
            m = NCORES * i + j
            if m >= MMAX:
                continue
            klo, span = windows[i]
            khi = min(klo + span, NLAT)
            n = khi - klo
            xfb[i, :n, :C] = xfr[:, klo:khi, m].T
            xfb[i, :n, C:] = xfi[:, klo:khi, m].T
            wtj[i, :n] = wtf[m, klo:khi]
        # repack rows (t*128+p) -> [p, (t f)] for contiguous multi-KB DMA lines
        xfb = np.ascontiguousarray(
            xfb.reshape(MPC, KPAD // 128, 128, 2 * C).transpose(0, 2, 1, 3)
        ).reshape(MPC, 128, (KPAD // 128) * 2 * C)
        wtj = np.ascontiguousarray(
            wtj.reshape(MPC, KPAD // 128, 128, LMAX).transpose(0, 2, 1, 3)
        ).reshape(MPC, 128, (KPAD // 128) * LMAX)
        in_maps_b.append({"xfb": xfb, "wt": wtj})
    nc_b = build_stage_b(nkc_list=nkc_list)
    res_b = _run(nc_b, in_maps_b, "stage_b")

    out = np.zeros((1, C, LMAX, MMAX), dtype=np.complex64)
    for j in range(NCORES):
        ml = m_list(j)
        o = np.asarray(res_b.results[j]["out"][: len(ml)], dtype=np.float32)
        out[0][:, :, ml] = (o[:, :, :C] + 1j * o[:, :, C:]).transpose(2, 1, 0)
    return out



# revision 11
# speedup vs baseline: 2.0776x; 1.0149x over previous
"""Distributed real SHT (spherical harmonic transform) on 8 trn2 NeuronCores.

Pipeline:
  out[b,c,l,m] = sum_k W[m,l,k] * XF[b,c,m,k],   XF = (2*pi/nlon) * rfft(x, lon)[..., :mmax]

Stage A (launch 1, channel-sharded): DFT along longitude as bf16 matmuls.
  Host folds x over lon parity (cos: n'=0..360, sin: n'=1..359), transposes to
  [c, n', k] (contraction n' on partitions), pads n' chunks to 3x128 per parity
  and k to 362, packs cos+sin into one tensor so each channel loads with ONE DMA.
  psum[k_tile, m] += xT[n'chunk, k_tile]^T @ DFTmat[n'chunk, m]
Host exchange: XF[c,k,m] (channel-sharded) -> XFB[m_local, k, (ri,c)] (m-sharded).
Stage B (launch 2, m-sharded, m interleaved mod 8 for triangular balance):
  psum[l_tile, 512] += WT[m][k, l_tile]^T @ XFB[m][k, (ri,c)=512]
  Only l >= m is computed (weights are exactly zero below the diagonal).

bf16 operands keep the PE at 2.4 GHz (fp32r matmuls don't count as HAM activity
and pin the array at 1.2 GHz) and halve DMA bytes; psum accumulation is fp32.
k is padded to 384=3x128 so rhs/weight loads are one rearranged DMA per tile.
"""

import os

import numpy as np

import concourse.bacc as bacc
import concourse.mybir as mybir
from concourse.tile import TileContext
from concourse.bass_utils import run_bass_kernel_spmd

LAST_PERF = {}

NLAT = 361
NLON = 720
MMAX = 361
LMAX = 361
C = 256
NCORES = 8
CPC = C // NCORES  # 32 channels per core
NC_COS = NLON // 2 + 1  # 361 cos columns (n'=0..360)
NC_SIN = NLON // 2 - 1  # 359 sin columns (n'=1..359)
MPC = (MMAX + NCORES - 1) // NCORES  # 46 m's per core (padded)
KPAD = 384  # nlat padded to 3x128 partition chunks
NPAD = 384  # per-parity n' padded to 3x128
MEVEN = 362  # k (stage A moving free dim) padded even

F32 = mybir.dt.float32
BF16 = mybir.dt.bfloat16


def _ptiles(n, p=128):
    out = []
    o = 0
    while o < n:
        out.append((o, min(p, n - o)))
        o += p
    return out


def build_stage_a(cpc=CPC, nlat=NLAT, mmax=MMAX):
    """Inputs: xin [cpc, 2*NPAD, MEVEN] bf16 (cos rows 0:361, sin rows 384:743,
    both zero-padded; last col zero), mats [2*NPAD, MEVEN] bf16 (same row
    packing; DFT matrices with the 2*pi/nlon scale; col dim is m padded even).
    Outputs: xfr/xfi [cpc, KPAD, mmax] bf16 (k rows >= 361 are garbage)."""
    nc = bacc.Bacc("TRN2", target_bir_lowering=False)
    nseg0 = 2 * NPAD // 128
    xin = nc.dram_tensor("xin", [cpc, 128, nseg0 * MEVEN], BF16, kind="ExternalInput")
    mats = nc.dram_tensor("mats", [128, nseg0 * MEVEN], BF16, kind="ExternalInput")
    xfr = nc.dram_tensor("xfr", [cpc, 128, (KPAD // 128) * mmax], BF16, kind="ExternalOutput")
    xfi = nc.dram_tensor("xfi", [cpc, 128, (KPAD // 128) * mmax], BF16, kind="ExternalOutput")

    nseg = 2 * NPAD // 128  # 6 partition segments: 3 cos + 3 sin
    k_tiles = _ptiles(nlat)  # psum partition tiles over k (128,128,105)
    copy_idx = 0
    with TileContext(nc) as tc:
        with (
            tc.tile_pool(name="mats", bufs=1) as matp,
            tc.tile_pool(name="xinp", bufs=5) as xinp,
            tc.tile_pool(name="outp", bufs=4) as outp,
            tc.tile_pool(name="ps", bufs=7, space="PSUM") as psp,
        ):
            mat_t = matp.tile([128, nseg * MEVEN], BF16, tag="mats")
            nc.sync.dma_start(out=mat_t, in_=mats[:, :])

            for c in range(cpc):
                x_t = xinp.tile([128, nseg * MEVEN], BF16, tag="xin")
                nc.sync.dma_start(out=x_t, in_=xin[c])
                for ri, odram in enumerate((xfr, xfi)):
                    ot = outp.tile([128, len(k_tiles) * mmax], BF16, tag="ot")
                    kp_last = k_tiles[-1][1]
                    # slots are reused round-robin; after the first 4 fills the
                    # pad rows hold stale-but-defined data the host ignores
                    if kp_last < 128 and c < 2:
                        base = (kp_last // 32) * 32  # partition offsets must be 32-aligned
                        nc.gpsimd.memset(
                            ot[base:, (len(k_tiles) - 1) * mmax :], 0.0
                        )
                    for kt, (k0, kp) in enumerate(k_tiles):
                        ps = psp.tile([128, mmax + 1], F32, tag="ps")
                        for s in range(3):
                            seg = 3 * ri + s
                            nc.tensor.matmul(
                                ps[:kp, :],
                                x_t[:, seg * MEVEN + k0 : seg * MEVEN + k0 + kp],
                                mat_t[:, seg * MEVEN : seg * MEVEN + mmax + 1],
                                start=(s == 0),
                                stop=(s == 2),
                            )
                        dst = ot[:kp, kt * mmax : (kt + 1) * mmax]
                        if copy_idx % 2 == 0:
                            nc.vector.tensor_copy(out=dst, in_=ps[:kp, :mmax])
                        else:
                            nc.scalar.copy(dst, ps[:kp, :mmax])
                        copy_idx += 1
                    nc.gpsimd.dma_start(out=odram[c], in_=ot)
    nc.compile()
    return nc


def build_stage_b(mpc=MPC, nlat=NLAT, lmax=LMAX, ncores=NCORES, nkc_list=None):
    """Inputs: xfb [mpc, KPAD, 512] bf16, wt [mpc, KPAD, lmax] bf16 -> out
    [mpc, lmax, 512] bf16. Index i handles m = ncores*i + core_j; computes l in
    [ncores*i, lmax) uniformly across cores (weights are zero for l < m ->
    exact zeros). nkc_list[i] gives the number of 128-row k chunks actually
    contracted for index i (the host packs only the latitude window where
    P_l^m is non-negligible -- it shrinks toward the equator as m grows)."""
    nc = bacc.Bacc("TRN2", target_bir_lowering=False)
    nric = 2 * C
    xfb = nc.dram_tensor("xfb", [mpc, 128, (KPAD // 128) * nric], BF16, kind="ExternalInput")
    wt = nc.dram_tensor("wt", [mpc, 128, (KPAD // 128) * lmax], BF16, kind="ExternalInput")
    # bf16 output halves the store wire; the host upcasts when assembling
    out = nc.dram_tensor("out", [mpc, lmax, nric], BF16, kind="ExternalOutput")

    nkc_max = KPAD // 128
    if nkc_list is None:
        nkc_list = [nkc_max] * mpc
    order = b_order(mpc)
    with TileContext(nc) as tc:
        with (
            tc.tile_pool(name="rhs", bufs=10) as rhsp,
            tc.tile_pool(name="wts", bufs=10) as wtp,
            tc.tile_pool(name="outp", bufs=10) as outp,
            tc.tile_pool(name="ps", bufs=7, space="PSUM") as psp,
        ):
            for bi in range(mpc):
                i = order[bi]  # buffer bi holds data for logical index i
                nkc = nkc_list[i]
                rhs_t = rhsp.tile([128, nkc_max * nric], BF16, tag="rhs")
                eng_a = nc.sync if bi % 2 == 0 else nc.scalar
                eng_b = nc.scalar if bi % 2 == 0 else nc.sync
                eng_a.dma_start(
                    out=rhs_t.rearrange("p (t f) -> p t f", t=nkc_max)[:, :nkc],
                    in_=xfb[i].rearrange("p (t f) -> p t f", t=nkc_max)[:, :nkc],
                )
                l_lo = ncores * i
                w_t = wtp.tile([128, nkc_max * lmax], BF16, tag="wt")
                # opposite HWDGE ring from rhs; only the l >= l_lo triangle
                eng_b.dma_start(
                    out=w_t.rearrange("p (t l) -> p t l", t=nkc_max)[:, :nkc, l_lo:],
                    in_=wt[i].rearrange("p (t l) -> p t l", t=nkc_max)[:, :nkc, l_lo:],
                )
                for l0, lp in _ptiles(lmax - l_lo):
                    la = l_lo + l0
                    ps = psp.tile([128, nric], F32, tag="ps")
                    for kc in range(nkc):
                        nc.tensor.matmul(
                            ps[:lp, :],
                            w_t[:, kc * lmax + la : kc * lmax + la + lp],
                            rhs_t[:, kc * nric : (kc + 1) * nric],
                            start=(kc == 0),
                            stop=(kc == nkc - 1),
                        )
                    ot = outp.tile([128, nric], BF16, tag="ot")
                    nc.vector.tensor_copy(out=ot[:lp, :], in_=ps[:lp, :])
                    nc.gpsimd.dma_start(out=out[i, la : la + lp, :], in_=ot[:lp, :])
    nc.compile()
    return nc


def _dft_matrices():
    """cosm[n', m] = s*cos(2 pi m n'/nlon), n'=0..360
    sinm[n', m] = -s*sin(2 pi m n'/nlon), n'=1..359 (imag of rfft = -sum x sin)."""
    s = 2.0 * np.pi / NLON
    m = np.arange(MMAX)
    nc_ = np.arange(NC_COS)
    ns_ = np.arange(1, NLON // 2)
    ang_c = 2.0 * np.pi * ((nc_[:, None] * m[None, :]) % NLON) / NLON
    ang_s = 2.0 * np.pi * ((ns_[:, None] * m[None, :]) % NLON) / NLON
    return (s * np.cos(ang_c)).astype(np.float32), (-s * np.sin(ang_s)).astype(
        np.float32
    )


def fold_x(x):
    """x: (C, nlat, nlon) f32 -> xc (C, nlat, 361), xs (C, nlat, 359)."""
    xc = np.empty((x.shape[0], x.shape[1], NC_COS), dtype=np.float32)
    xc[..., 0] = x[..., 0]
    xc[..., NLON // 2] = x[..., NLON // 2]
    xc[..., 1 : NLON // 2] = x[..., 1 : NLON // 2] + x[..., : NLON // 2 : -1]
    xs = x[..., 1 : NLON // 2] - x[..., : NLON // 2 : -1]
    return xc, np.ascontiguousarray(xs.astype(np.float32))


def pack_stage_a_inputs(x):
    """x: (C, nlat, nlon) f32 -> xin (C, 768, 362) bf16, mats (768, 362) bf16."""
    import ml_dtypes

    bf = ml_dtypes.bfloat16
    xc, xs = fold_x(x)
    xin = np.zeros((x.shape[0], 2 * NPAD, MEVEN), dtype=bf)
    xin[:, :NC_COS, :NLAT] = xc.transpose(0, 2, 1).astype(bf)
    xin[:, NPAD : NPAD + NC_SIN, :NLAT] = xs.transpose(0, 2, 1).astype(bf)
    cosm, sinm = _dft_matrices()
    mats = np.zeros((2 * NPAD, MEVEN), dtype=bf)
    mats[:NC_COS, :MMAX] = cosm.astype(bf)
    mats[NPAD : NPAD + NC_SIN, :MMAX] = sinm.astype(bf)
    # repack rows (s*128+p) -> [p, (s k)] so each channel loads/stores with
    # single DMAs of ~4.3KB contiguous per-partition lines (DMA engines cost
    # ~9ns + bytes/27GB/s per line; 724B lines only reach ~60% of peak)
    nseg = 2 * NPAD // 128
    xin = np.ascontiguousarray(
        xin.reshape(x.shape[0], nseg, 128, MEVEN).transpose(0, 2, 1, 3)
    ).reshape(x.shape[0], 128, nseg * MEVEN)
    mats = np.ascontiguousarray(
        mats.reshape(nseg, 128, MEVEN).transpose(1, 0, 2)
    ).reshape(128, nseg * MEVEN)
    return xin, mats


def b_order(mpc):
    """Interleave heavy (small i, 3 l-tiles) and light (large i) iterations;
    the lightest index runs last so the post-matmul drain tail is minimal."""
    order = []
    lo, hi = 0, mpc - 2
    while lo <= hi:
        order.append(lo)
        if hi != lo:
            order.append(hi)
        lo += 1
        hi -= 1
    order.append(mpc - 1)
    return order


def m_list(j):
    return [NCORES * i + j for i in range(MPC) if NCORES * i + j < MMAX]


def _install_ntff_hook():
    """This image's antenv lacks axon_hooks; synthesize it so bass_utils'
    trace=True path can capture NTFFs via the axon PJRT .so."""
    import sys

    if "antenv.axon_hooks" in sys.modules:
        return
    import types

    mod = types.ModuleType("antenv.axon_hooks")
    state = {"hook": None}
    mod.set_axon_ntff_profile_hook = lambda h: state.__setitem__("hook", h)
    mod.get_axon_ntff_profile_hook = lambda: state["hook"]
    sys.modules["antenv.axon_hooks"] = mod
    try:
        import importlib.util as ilu

        spec = ilu.spec_from_file_location(
            "_trn_boot_hook", "/root/.axon_site/trn_agent_boot/trn_boot.py"
        )
        tb = ilu.module_from_spec(spec)
        spec.loader.exec_module(tb)
        mod.set_axon_ntff_profile_hook(
            tb._ntff_profile_via_ctypes("/opt/axon/libaxon_pjrt.so")
        )
    except Exception:
        pass


def _run(nc, in_maps, label):
    kw = {}
    if os.environ.get("SHT_TRACE"):
        import concourse.bass_utils as bu

        bu.upload_artifacts = lambda tmpdir: tmpdir  # no S3 in this sandbox
        _install_ntff_hook()
        kw = dict(trace=True)
    try:
        res = run_bass_kernel_spmd(nc, in_maps, core_ids=list(range(NCORES)), **kw)
    except Exception:
        if not kw:
            raise
        res = run_bass_kernel_spmd(nc, in_maps, core_ids=list(range(NCORES)))
    LAST_PERF[label] = res.exec_time_ns
    return res


def kernel(x, weights):
    import ml_dtypes

    bf = ml_dtypes.bfloat16
    x = np.asarray(x, dtype=np.float32).reshape(C, NLAT, NLON)
    weights = np.asarray(weights, dtype=np.float32)

    xin, mats = pack_stage_a_inputs(x)
    nc_a = build_stage_a()
    in_maps = [
        {"xin": xin[j * CPC : (j + 1) * CPC], "mats": mats} for j in range(NCORES)
    ]
    res_a = _run(nc_a, in_maps, "stage_a")
    # [c, p, (t m)] -> (C, k=t*128+p, m), drop k padding rows
    nt = KPAD // 128
    xfr = np.concatenate(
        [np.asarray(r["xfr"]).reshape(-1, 128, nt, MMAX) for r in res_a.results], axis=0
    ).transpose(0, 2, 1, 3).reshape(C, KPAD, MMAX)[:, :NLAT, :]
    xfi = np.concatenate(
        [np.asarray(r["xfi"]).reshape(-1, 128, nt, MMAX) for r in res_a.results], axis=0
    ).transpose(0, 2, 1, 3).reshape(C, KPAD, MMAX)[:, :NLAT, :]

    wtf = weights.transpose(0, 2, 1).astype(bf)  # (m, k, l)
    # per-index latitude windows: union of |W| support over the 8 cores' m's
    wabs = np.abs(weights).max(axis=1)  # (m, k)
    thr = 1e-7 * wabs.max()
    windows = []
    for i in range(MPC):
        ms = [NCORES * i + j for j in range(NCORES) if NCORES * i + j < MMAX]
        nz = np.nonzero(wabs[ms].max(axis=0) > thr)[0]
        klo, khi = (int(nz[0]), int(nz[-1]) + 1) if len(nz) else (0, NLAT)
        span = min(-(-max(khi - klo, 1) // 128) * 128, KPAD)
        klo = max(0, min(klo, NLAT - span)) if span < NLAT else 0
        windows.append((klo, span))
    nkc_list = [span // 128 for _, span in windows]
    in_maps_b = []
    for j in range(NCORES):
        ml = m_list(j)
        xfb = np.zeros((MPC, KPAD, 2 * C), dtype=bf)
        wtj = np.zeros((MPC, KPAD, LMAX), dtype=bf)
        for i in range(MPC):
            m = NCORES * i + j
            if m >= MMAX:
                continue
            klo, span = windows[i]
            khi = min(klo + span, NLAT)
            n = khi - klo
            xfb[i, :n, :C] = xfr[:, klo:khi, m].T
            xfb[i, :n, C:] = xfi[:, klo:khi, m].T
            wtj[i, :n] = wtf[m, klo:khi]
        # repack rows (t*128+p) -> [p, (t f)] for contiguous multi-KB DMA lines
        xfb = np.ascontiguousarray(
            xfb.reshape(MPC, KPAD // 128, 128, 2 * C).transpose(0, 2, 1, 3)
        ).reshape(MPC, 128, (KPAD // 128) * 2 * C)
        wtj = np.ascontiguousarray(
            wtj.reshape(MPC, KPAD // 128, 128, LMAX).transpose(0, 2, 1, 3)
        ).reshape(MPC, 128, (KPAD // 128) * LMAX)
        in_maps_b.append({"xfb": xfb, "wt": wtj})
    nc_b = build_stage_b(nkc_list=nkc_list)
    res_b = _run(nc_b, in_maps_b, "stage_b")

    out = np.zeros((1, C, LMAX, MMAX), dtype=np.complex64)
    for j in range(NCORES):
        ml = m_list(j)
        o = np.asarray(res_b.results[j]["out"][: len(ml)], dtype=np.float32)
        out[0][:, :, ml] = (o[:, :, :C] + 1j * o[:, :, C:]).transpose(2, 1, 0)
    return out

